# revision 1
# baseline (speedup 1.0000x reference)
"""Trainium2 Bass kernel for a convolutional GRU (nn_ConvolutionalRNN).

Reference semantics (per timestep t, torch-GRUCell-style with conv1d gates):
    gi = conv1d(x[t], w_ih) + b_ih          # [B, 3C, L], precomputable
    gh = conv1d(h,    w_hh) + b_hh          # [B, 3C, L], recurrent
    r = sigmoid(gi_r + gh_r); z = sigmoid(gi_z + gh_z)
    n = tanh(gi_n + r * gh_n)
    h = n + z * (h - n)  =  z*h + n*(1-z)
    ys[t] = h

Sharding: data-parallel over batch. B=16 across 8 NeuronCores -> 2 batch
items per core; weights replicated; T stays local (sequential recurrence).

Per-core on-chip layout: channels on partitions, (b, l) on the free axis
(N=512 per matmul = one PSUM bank). The K=3 conv is 3 shifted matmuls
(contraction over CIN=64) accumulating in PSUM; the input-side conv is fused
into the same PSUM accumulation as the recurrent conv, so gi never touches
HBM. Matmuls run in float32r (fp32 data rounded to 12-bit mantissa ->
full-rate PE); everything else is fp32. The GRU update uses
h_new = z*h + n*sigmoid(-pre_z): z*h is computed off the critical path, so
only two vector ops trail the tanh.
"""

import numpy as np
from contextlib import ExitStack

from concourse import bacc, mybir
import concourse.tile as tile
from concourse.bass_utils import run_bass_kernel_spmd

T, B, CIN, COUT, L = 128, 16, 64, 64, 256
GATES = 3 * COUT
NCORES = 8
BL = B // NCORES          # batch per core = 2
LP = L + 2                # padded length (zero border at l=0 and l=L+1)
F32 = mybir.dt.float32
F32R = mybir.dt.float32r
AF = mybir.ActivationFunctionType
ALU = mybir.AluOpType


def _round_fp32r(x: np.ndarray) -> np.ndarray:
    """Round fp32 to the fp32r grid (12-bit mantissa, round-nearest-even) —
    matches what TRN2 produces when an engine writes a float32r output."""
    u = np.ascontiguousarray(x, np.float32).view(np.uint32).copy()
    low = u & np.uint32(0xFFF)
    u &= np.uint32(0xFFFFF000)
    up = (low > 0x800) | ((low == 0x800) & (((u >> 12) & 1) == 1))
    u[up] += np.uint32(0x1000)
    return u.view(np.float32)


def _build_nc():
    nc = bacc.Bacc(trn_type="TRN2", target_bir_lowering=False, debug=False)

    # Per-core DRAM I/O. Host pre-transposes to channel-major so every DMA
    # is 2 KB-contiguous per partition.
    x_d = nc.dram_tensor("x", [T, CIN, BL, L], F32R, kind="ExternalInput").ap()
    h0_d = nc.dram_tensor("h0", [COUT, BL, L], F32R, kind="ExternalInput").ap()
    wih_d = nc.dram_tensor("wih", [CIN, 3, GATES], F32R, kind="ExternalInput").ap()
    whh_d = nc.dram_tensor("whh", [CIN, 3, GATES], F32R, kind="ExternalInput").ap()
    brz_d = nc.dram_tensor("brz", [2 * COUT, 1], F32, kind="ExternalInput").ap()
    bzn_d = nc.dram_tensor("bzn", [COUT, 1], F32, kind="ExternalInput").ap()
    bihn_d = nc.dram_tensor("bihn", [COUT, 1], F32, kind="ExternalInput").ap()
    bhhn_d = nc.dram_tensor("bhhn", [COUT, 1], F32, kind="ExternalInput").ap()
    ys_d = nc.dram_tensor("ys", [T, COUT, BL, L], F32, kind="ExternalOutput").ap()

    NB = 2  # x_buf double-buffer depth

    with tile.TileContext(nc) as tc, ExitStack() as ctx:
        persist = ctx.enter_context(tc.tile_pool(name="persist", bufs=1))
        work = ctx.enter_context(tc.tile_pool(name="work", bufs=3))
        psA = ctx.enter_context(tc.tile_pool(name="psA", bufs=2, space="PSUM"))
        psB1 = ctx.enter_context(tc.tile_pool(name="psB1", bufs=2, space="PSUM"))
        psB2 = ctx.enter_context(tc.tile_pool(name="psB2", bufs=2, space="PSUM"))

        # --- one-time setup -------------------------------------------------
        wih = persist.tile([CIN, 3, GATES], F32)
        whh = persist.tile([CIN, 3, GATES], F32)
        nc.sync.dma_start(out=wih[:].bitcast(F32R), in_=wih_d)
        nc.sync.dma_start(out=whh[:].bitcast(F32R), in_=whh_d)
        brz = persist.tile([2 * COUT, 1], F32)
        bzn = persist.tile([COUT, 1], F32)
        bihn = persist.tile([COUT, 1], F32)
        bhhn = persist.tile([COUT, 1], F32)
        nc.sync.dma_start(out=brz[:], in_=brz_d)
        nc.sync.dma_start(out=bzn[:], in_=bzn_d)
        nc.sync.dma_start(out=bihn[:], in_=bihn_d)
        nc.sync.dma_start(out=bhhn[:], in_=bhhn_d)

        h_buf = persist.tile([COUT, BL, LP], F32)
        x_buf = persist.tile([CIN, NB, BL, LP], F32)
        nc.vector.memset(h_buf[:], 0.0)
        nc.vector.memset(x_buf[:], 0.0)
        nc.sync.dma_start(out=h_buf[:, :, 1:L + 1].bitcast(F32R), in_=h0_d)

        h_in = h_buf[:, :, 1:L + 1]            # [COUT, BL, L] interior view

        def mm(out_ps, w_tile, g0, g1, rhs_buf, tap, start, stop):
            """out_ps += w[:, tap, g0:g1]^T @ rhs_buf shifted by tap."""
            nc.tensor.matmul(
                out_ps,
                w_tile[:, tap, g0:g1].bitcast(F32R),
                rhs_buf[:, :, tap:tap + L].bitcast(F32R),
                start=start, stop=stop,
            )

        # --- the recurrence -------------------------------------------------
        for t in range(T):
            xb = x_buf[:, t % NB]              # [CIN, BL, LP]
            nc.sync.dma_start(out=xb[:, :, 1:L + 1].bitcast(F32R), in_=x_d[t])

            pre_rz = psA.tile([2 * COUT, BL, L], F32)   # i_r+h_r | i_z+h_z
            i_n = psB1.tile([COUT, BL, L], F32)
            gh_n = psB2.tile([COUT, BL, L], F32)

            # input-side convs (off critical path; only need x[t])
            for k in range(3):
                mm(pre_rz[:], wih, 0, 128, xb, k, start=(k == 0), stop=False)
            for k in range(3):
                mm(i_n[:], wih, 128, 192, xb, k, start=(k == 0), stop=(k == 2))
            # recurrent convs (critical path; need h(t-1))
            for k in range(3):
                mm(pre_rz[:], whh, 0, 128, h_buf, k, start=False, stop=(k == 2))
            for k in range(3):
                mm(gh_n[:], whh, 128, 192, h_buf, k, start=(k == 0), stop=(k == 2))

            r = work.tile([COUT, BL, L], F32, tag="r")
            z = work.tile([COUT, BL, L], F32, tag="z")
            zc = work.tile([COUT, BL, L], F32, tag="zc")
            nc.scalar.activation(r[:], pre_rz[0:COUT], AF.Sigmoid, bias=brz[0:COUT])
            nc.scalar.activation(z[:], pre_rz[COUT:2 * COUT], AF.Sigmoid,
                                 bias=brz[COUT:2 * COUT])
            nc.scalar.activation(zc[:], pre_rz[COUT:2 * COUT], AF.Sigmoid,
                                 bias=bzn[:], scale=-1.0)

            # t1 = (gh_n + b_hhn) * r ; t2 = t1 + i_n ; n = tanh(t2 + b_ihn)
            t1 = work.tile([COUT, BL, L], F32, tag="t1")
            nc.vector.scalar_tensor_tensor(t1[:], gh_n[:], bhhn[:], r[:],
                                           op0=ALU.add, op1=ALU.mult)
            t2 = work.tile([COUT, BL, L], F32, tag="t2")
            nc.vector.tensor_add(t2[:], t1[:], i_n[:])
            n = work.tile([COUT, BL, L], F32, tag="n")
            nc.scalar.activation(n[:], t2[:], AF.Tanh, bias=bihn[:])

            # h_new = z*h + n*zc   (z*h runs before tanh finishes)
            zh = work.tile([COUT, BL, L], F32, tag="zh")
            nc.vector.tensor_mul(zh[:], z[:], h_in)
            nzc = work.tile([COUT, BL, L], F32, tag="nzc")
            nc.vector.tensor_mul(nzc[:], n[:], zc[:])
            nc.vector.tensor_add(h_in.bitcast(F32R), zh[:], nzc[:])

            nc.sync.dma_start(out=ys_d[t], in_=h_in)

    nc.compile()
    return nc


_NC = None


def _get_nc():
    global _NC
    if _NC is None:
        _NC = _build_nc()
    return _NC


def _prep_in_maps(x, h0, w_ih, w_hh, b_ih, b_hh):
    # weights: [GATES, CIN, K] -> [CIN, K, GATES], fp32r-rounded
    wih_t = _round_fp32r(np.transpose(np.asarray(w_ih, np.float32), (1, 2, 0)))
    whh_t = _round_fp32r(np.transpose(np.asarray(w_hh, np.float32), (1, 2, 0)))
    b_ih = np.asarray(b_ih, np.float32)
    b_hh = np.asarray(b_hh, np.float32)
    brz = (b_ih[:2 * COUT] + b_hh[:2 * COUT]).reshape(2 * COUT, 1)
    bzn = -brz[COUT:2 * COUT]
    bihn = b_ih[2 * COUT:].reshape(COUT, 1)
    bhhn = b_hh[2 * COUT:].reshape(COUT, 1)

    x = _round_fp32r(np.asarray(x, np.float32))
    h0 = _round_fp32r(np.asarray(h0, np.float32))
    in_maps = []
    for c in range(NCORES):
        xs = np.ascontiguousarray(
            np.transpose(x[:, c * BL:(c + 1) * BL], (0, 2, 1, 3)))
        h0s = np.ascontiguousarray(
            np.transpose(h0[c * BL:(c + 1) * BL], (1, 0, 2)))
        in_maps.append({
            "x": xs, "h0": h0s, "wih": wih_t, "whh": whh_t,
            "brz": brz, "bzn": bzn, "bihn": bihn, "bhhn": bhhn,
        })
    return in_maps


def kernel(x, h0, w_ih, w_hh, b_ih, b_hh):
    nc = _get_nc()
    in_maps = _prep_in_maps(x, h0, w_ih, w_hh, b_ih, b_hh)
    res = run_bass_kernel_spmd(nc, in_maps, list(range(NCORES)))
    ys = np.empty((T, B, COUT, L), np.float32)
    for c in range(NCORES):
        ys[:, c * BL:(c + 1) * BL] = np.transpose(
            res.results[c]["ys"], (0, 2, 1, 3))
    return ys



# revision 4
# speedup vs baseline: 1.2449x; 1.2449x over previous
"""Trainium2 Bass kernel for a convolutional GRU (nn_ConvolutionalRNN).

Reference semantics (per timestep t, torch-GRUCell-style with conv1d gates):
    gi = conv1d(x[t], w_ih) + b_ih          # [B, 3C, L], precomputable
    gh = conv1d(h,    w_hh) + b_hh          # [B, 3C, L], recurrent
    r = sigmoid(gi_r + gh_r); z = sigmoid(gi_z + gh_z)
    n = tanh(gi_n + r * gh_n)
    h = n + z * (h - n)  =  z*h + n*(1-z)
    ys[t] = h

Sharding: data-parallel over batch: B=16 across 8 cores -> BL=2 rows/core.
The two batch rows per core are INDEPENDENT recurrences; they are run as
two skewed software pipelines (chains) so engine work of row 1 fills the
dependency stalls of row 0.

v2 design (vs fp32r baseline at ~950us):
 - fp16 everywhere on chip (PE fp16 = 1 cyc/row always; DVE 16-bit 2x
   modes; x and ys live in HBM as fp16 - host converts, halving DMA).
 - r and z computed in ONE sigmoid over 128 partitions (bias port takes
   the per-partition brz vector).
 - biases folded into free ports: brz -> sigmoid bias, bihn -> tanh bias,
   bhhn -> scalar_tensor_tensor scalar operand. No bias adds anywhere.
 - input-side conv taps 0,1 K-packed to 128 partitions (x is DMA'd twice,
   the second copy shifted by one column) -> 2 MMs instead of 3.
 - per-(t,b) PSUM banks: bankA = pre_rz [128,256], bankB = [gh_n; i_n]
   [128,256] so the skewed chains never share a PSUM bank.
 - z*h runs on GPSIMD (idle engine), zc=1-z on the DVE in the tanh wait
   slot; the critical DVE chain is stt(t1) -> add(t2) -> mul -> add.
 - 32 dummy matmuls at kernel start warm the PE HAM clock gate to 2.4GHz
   (the baseline ran 100% of matmuls at the cold 1.2GHz rate).
"""

import numpy as np
from contextlib import ExitStack

from concourse import bacc, mybir
import concourse.tile as tile
from concourse.bass_utils import run_bass_kernel_spmd

T, B, CIN, COUT, L = 128, 16, 64, 64, 256
GATES = 3 * COUT
NCORES = 8
BL = B // NCORES          # batch rows per core = 2
LP = L + 2                # padded length (zero border at col 0 and L+1)
NB = 3                    # x buffer depth (steps of DMA lookahead)
F32 = mybir.dt.float32
F16 = mybir.dt.float16
AF = mybir.ActivationFunctionType
ALU = mybir.AluOpType


def _build_nc():
    nc = bacc.Bacc(trn_type="TRN2", target_bir_lowering=False, debug=False)

    # Per-core DRAM I/O (fp16 data; fp32 biases).
    x_d = nc.dram_tensor("x", [T, CIN, BL, L], F16, kind="ExternalInput").ap()
    h0_d = nc.dram_tensor("h0", [COUT, BL, L], F16, kind="ExternalInput").ap()
    wprz_d = nc.dram_tensor("wprz", [2 * CIN, 2 * COUT], F16, kind="ExternalInput").ap()
    w2rz_d = nc.dram_tensor("w2rz", [CIN, 2 * COUT], F16, kind="ExternalInput").ap()
    wpn_d = nc.dram_tensor("wpn", [2 * CIN, COUT], F16, kind="ExternalInput").ap()
    w2n_d = nc.dram_tensor("w2n", [CIN, COUT], F16, kind="ExternalInput").ap()
    whhrz_d = nc.dram_tensor("whhrz", [COUT, 3, 2 * COUT], F16, kind="ExternalInput").ap()
    whhn_d = nc.dram_tensor("whhn", [COUT, 3, COUT], F16, kind="ExternalInput").ap()
    brz_d = nc.dram_tensor("brz", [2 * COUT, 1], F32, kind="ExternalInput").ap()
    bihn_d = nc.dram_tensor("bihn", [COUT, 1], F32, kind="ExternalInput").ap()
    bhhn_d = nc.dram_tensor("bhhn", [COUT, 1], F32, kind="ExternalInput").ap()
    ys_d = nc.dram_tensor("ys", [T, COUT, BL, L], F16, kind="ExternalOutput").ap()

    with tile.TileContext(nc) as tc, ExitStack() as ctx:
        persist = ctx.enter_context(tc.tile_pool(name="persist", bufs=1))
        work = ctx.enter_context(tc.tile_pool(name="work", bufs=2))
        psA = [ctx.enter_context(tc.tile_pool(name=f"psA{b}", bufs=2, space="PSUM"))
               for b in range(BL)]
        psB = [ctx.enter_context(tc.tile_pool(name=f"psB{b}", bufs=2, space="PSUM"))
               for b in range(BL)]

        # --- one-time setup -------------------------------------------------
        # Input-side weights, taps 0+1 K-packed (rows 0-63 tap0, 64-127 tap1).
        wprz = persist.tile([2 * CIN, 2 * COUT], F16)
        w2rz = persist.tile([CIN, 2 * COUT], F16)
        wpn = persist.tile([2 * CIN, COUT], F16)
        w2n = persist.tile([CIN, COUT], F16)
        # Recurrent weights live on partitions 64-127 (h lives there too, so
        # lhsT/rhs base partitions match; frees partitions 0-63 of the rhs
        # for other tiles and puts z*h on gpsimd cores 4-7).
        whhrz = persist.tile([2 * COUT, 3, 2 * COUT], F16)
        whhn = persist.tile([2 * COUT, 3, COUT], F16)
        for t_, d_ in ((wprz, wprz_d), (w2rz, w2rz_d), (wpn, wpn_d), (w2n, w2n_d)):
            nc.sync.dma_start(out=t_[:], in_=d_)
        nc.sync.dma_start(out=whhrz[COUT:2 * COUT], in_=whhrz_d)
        nc.sync.dma_start(out=whhn[COUT:2 * COUT], in_=whhn_d)

        brz = persist.tile([2 * COUT, 1], F32)
        bihn = persist.tile([COUT, 1], F32)
        bhhn = persist.tile([COUT, 1], F32)
        nc.sync.dma_start(out=brz[:], in_=brz_d)
        nc.sync.dma_start(out=bihn[:], in_=bihn_d)
        nc.sync.dma_start(out=bhhn[:], in_=bhhn_d)

        # h state, one tile per batch row, on partitions 64-127; interior
        # cols 1..L, zero halo at cols 0 and L+1.
        hb = [persist.tile([2 * COUT, LP], F16, tag=f"h{b}", name=f"h{b}")
              for b in range(BL)]
        for b in range(BL):
            nc.vector.memset(hb[b][:], 0.0)
            nc.sync.dma_start(out=hb[b][COUT:2 * COUT, 1:L + 1], in_=h0_d[:, b, :])

        # x buffers: rows 0-63 = x shifted right by 1 col (tap0 view),
        # rows 64-127 = x (tap1 view). Col 0 of rows 0-63 must stay zero.
        xbuf = persist.tile([2 * CIN, NB, BL, LP], F16)
        nc.vector.memset(xbuf[:], 0.0)

        def dma_x(t):
            xb = xbuf[:, t % NB]
            nc.sync.dma_start(out=xb[0:CIN, :, 1:L + 1], in_=x_d[t])
            nc.sync.dma_start(out=xb[CIN:2 * CIN, :, 0:L], in_=x_d[t])

        # --- PE warmup: ~32 dense matmuls flip the HAM gate to 2.4GHz ------
        warm = persist.tile([128, 256], F16)
        nc.vector.memset(warm[:], 0.0)
        wps = psA[0].tile([128, L], F32, tag="bankA0")
        for _ in range(32):
            nc.tensor.matmul(wps[:], warm[:, 0:128], warm[:, 0:L],
                             start=True, stop=True)

        for t in range(min(NB - 1, T)):
            dma_x(t)

        def wih_mms(t, b):
            """Input-side convs for (t, b): 4 MMs, independent of h."""
            xb = xbuf[:, t % NB]
            bankA = psA[b].tile([128, L], F32, tag=f"bankA{b}")
            bankB = psB[b].tile([128, L], F32, tag=f"bankB{b}")
            nc.tensor.matmul(bankA[:], wprz[:], xb[:, b, 0:L],
                             start=True, stop=False)
            nc.tensor.matmul(bankA[:], w2rz[:], xb[0:CIN, b, 2:LP],
                             start=False, stop=False)
            nc.tensor.matmul(bankB[COUT:2 * COUT], wpn[:], xb[:, b, 0:L],
                             start=True, stop=False)
            nc.tensor.matmul(bankB[COUT:2 * COUT], w2n[:], xb[0:CIN, b, 2:LP],
                             start=False, stop=True)
            return bankA, bankB

        def whh_mms(t, b, bankA, bankB):
            """Recurrent convs for (t, b): 6 MMs, need h(t-1)."""
            h = hb[b]
            for k in range(3):
                nc.tensor.matmul(bankA[:], whhrz[COUT:2 * COUT, k],
                                 h[COUT:2 * COUT, k:k + L],
                                 start=False, stop=(k == 2))
            for k in range(3):
                nc.tensor.matmul(bankB[0:COUT], whhn[COUT:2 * COUT, k],
                                 h[COUT:2 * COUT, k:k + L],
                                 start=(k == 0), stop=(k == 2))

        # Prime: input convs for step 0.
        banks = [[None, None] for _ in range(T)]
        for b in range(BL):
            banks[0][b] = wih_mms(0, b)

        for t in range(T):
            if t + NB - 1 < T:
                dma_x(t + NB - 1)
            for b in range(BL):
                bankA, bankB = banks[t][b]
                whh_mms(t, b, bankA, bankB)
                # Prefetch next step's input convs while this chain's
                # elementwise tail runs (keeps the PE dense and warm).
                if t + 1 < T:
                    banks[t + 1][b] = wih_mms(t + 1, b)

                # The whole elementwise tail lives on partitions 64-127 so
                # every SBUF+SBUF operand pair shares a base partition (BIR
                # verifier requirement) and gpsimd work lands on cores 4-7.
                h = hb[b]
                rz = work.tile([2 * COUT, L], F16, tag=f"rz{b}")
                nc.scalar.activation(rz[:], bankA[:], AF.Sigmoid, bias=brz[:])
                # t1 = (gh_n + bhhn) * r ; t2 = t1 + i_n ; n = tanh(t2 + bihn)
                t1 = work.tile([2 * COUT, L], F16, tag=f"t1{b}")
                nc.vector.scalar_tensor_tensor(t1[COUT:2 * COUT],
                                               bankB[0:COUT], bhhn[:],
                                               rz[0:COUT], op0=ALU.add,
                                               op1=ALU.mult)
                t2 = work.tile([2 * COUT, L], F16, tag=f"t2{b}")
                nc.vector.tensor_add(t2[COUT:2 * COUT], t1[COUT:2 * COUT],
                                     bankB[COUT:2 * COUT])
                n = work.tile([2 * COUT, L], F16, tag=f"n{b}")
                nc.scalar.activation(n[COUT:2 * COUT], t2[COUT:2 * COUT],
                                     AF.Tanh, bias=bihn[:])
                # zc = 1 - z on DVE (fills the tanh wait slot); zh = z*h on
                # GPSIMD (cores 4-7; all operands on partitions 64-127).
                zc = work.tile([2 * COUT, L], F16, tag=f"zc{b}")
                nc.vector.tensor_scalar(zc[COUT:2 * COUT],
                                        rz[COUT:2 * COUT], -1.0, 1.0,
                                        op0=ALU.mult, op1=ALU.add)
                zh = work.tile([2 * COUT, L], F16, tag=f"zh{b}")
                nc.gpsimd.tensor_mul(zh[COUT:2 * COUT], rz[COUT:2 * COUT],
                                     h[COUT:2 * COUT, 1:L + 1])
                # h_new = z*h + n*zc
                nzc = work.tile([2 * COUT, L], F16, tag=f"nzc{b}")
                nc.vector.tensor_mul(nzc[COUT:2 * COUT], n[COUT:2 * COUT],
                                     zc[COUT:2 * COUT])
                nc.vector.tensor_add(h[COUT:2 * COUT, 1:L + 1],
                                     zh[COUT:2 * COUT], nzc[COUT:2 * COUT])
                nc.sync.dma_start(out=ys_d[t, :, b, :],
                                  in_=h[COUT:2 * COUT, 1:L + 1])

    nc.compile()
    return nc


_NC = None


def _get_nc():
    global _NC
    if _NC is None:
        _NC = _build_nc()
    return _NC


def _prep_in_maps(x, h0, w_ih, w_hh, b_ih, b_hh):
    w_ih = np.asarray(w_ih, np.float32)   # [GATES, CIN, 3]
    w_hh = np.asarray(w_hh, np.float32)
    b_ih = np.asarray(b_ih, np.float32)
    b_hh = np.asarray(b_hh, np.float32)

    # lhsT layouts: [K, M] with K = input-channel rows, M = gate cols.
    def packed(w, g0, g1):   # taps 0,1 stacked on K
        return np.concatenate(
            [np.transpose(w[g0:g1, :, 0], (1, 0)),
             np.transpose(w[g0:g1, :, 1], (1, 0))], axis=0).astype(np.float16)

    def tap2(w, g0, g1):
        return np.ascontiguousarray(
            np.transpose(w[g0:g1, :, 2], (1, 0))).astype(np.float16)

    wprz = packed(w_ih, 0, 2 * COUT)
    wpn = packed(w_ih, 2 * COUT, GATES)
    w2rz = tap2(w_ih, 0, 2 * COUT)
    w2n = tap2(w_ih, 2 * COUT, GATES)
    # Recurrent weights per tap: [COUT, 3, M]
    whhrz = np.ascontiguousarray(
        np.transpose(w_hh[0:2 * COUT], (1, 2, 0))).astype(np.float16)
    whhn = np.ascontiguousarray(
        np.transpose(w_hh[2 * COUT:GATES], (1, 2, 0))).astype(np.float16)

    brz = (b_ih[:2 * COUT] + b_hh[:2 * COUT]).reshape(2 * COUT, 1)
    bihn = b_ih[2 * COUT:].reshape(COUT, 1)
    bhhn = b_hh[2 * COUT:].reshape(COUT, 1)

    x = np.asarray(x, np.float32).astype(np.float16)
    h0 = np.asarray(h0, np.float32).astype(np.float16)
    in_maps = []
    for c in range(NCORES):
        xs = np.ascontiguousarray(
            np.transpose(x[:, c * BL:(c + 1) * BL], (0, 2, 1, 3)))
        h0s = np.ascontiguousarray(
            np.transpose(h0[c * BL:(c + 1) * BL], (1, 0, 2)))
        in_maps.append({
            "x": xs, "h0": h0s,
            "wprz": wprz, "w2rz": w2rz, "wpn": wpn, "w2n": w2n,
            "whhrz": whhrz, "whhn": whhn,
            "brz": brz, "bihn": bihn, "bhhn": bhhn,
        })
    return in_maps


def kernel(x, h0, w_ih, w_hh, b_ih, b_hh):
    nc = _get_nc()
    in_maps = _prep_in_maps(x, h0, w_ih, w_hh, b_ih, b_hh)
    res = run_bass_kernel_spmd(nc, in_maps, list(range(NCORES)))
    ys = np.empty((T, B, COUT, L), np.float32)
    for c in range(NCORES):
        ys[:, c * BL:(c + 1) * BL] = np.transpose(
            res.results[c]["ys"].astype(np.float32), (0, 2, 1, 3))
    return ys


# revision 7
# speedup vs baseline: 1.2625x; 1.0141x over previous
"""Trainium2 Bass kernel for a convolutional GRU (nn_ConvolutionalRNN).

Reference semantics (per timestep t, torch-GRUCell-style with conv1d gates):
    gi = conv1d(x[t], w_ih) + b_ih          # [B, 3C, L], precomputable
    gh = conv1d(h,    w_hh) + b_hh          # [B, 3C, L], recurrent
    r = sigmoid(gi_r + gh_r); z = sigmoid(gi_z + gh_z)
    n = tanh(gi_n + r * gh_n)
    h = n + z * (h - n)  =  z*h + n*(1-z)
    ys[t] = h

Sharding: data-parallel over batch: B=16 across 8 cores -> BL=2 rows/core.
The two batch rows per core are INDEPENDENT recurrences; they are run as
two skewed software pipelines (chains) so engine work of row 1 fills the
dependency stalls of row 0.

v2 design (vs fp32r baseline at ~950us):
 - fp16 everywhere on chip (PE fp16 = 1 cyc/row always; DVE 16-bit 2x
   modes; x and ys live in HBM as fp16 - host converts, halving DMA).
 - r and z computed in ONE sigmoid over 128 partitions (bias port takes
   the per-partition brz vector).
 - biases folded into free ports: brz -> sigmoid bias, bihn -> tanh bias,
   bhhn -> scalar_tensor_tensor scalar operand. No bias adds anywhere.
 - input-side conv taps 0,1 K-packed to 128 partitions (x is DMA'd twice,
   the second copy shifted by one column) -> 2 MMs instead of 3.
 - per-(t,b) PSUM banks: bankA = pre_rz [128,256], bankB = [gh_n; i_n]
   [128,256] so the skewed chains never share a PSUM bank.
 - z*h runs on GPSIMD (idle engine), zc=1-z on the DVE in the tanh wait
   slot; the critical DVE chain is stt(t1) -> add(t2) -> mul -> add.
 - 32 dummy matmuls at kernel start warm the PE HAM clock gate to 2.4GHz
   (the baseline ran 100% of matmuls at the cold 1.2GHz rate).
"""

import numpy as np
from contextlib import ExitStack

from concourse import bacc, mybir
import concourse.tile as tile
from concourse.bass_utils import run_bass_kernel_spmd

T, B, CIN, COUT, L = 128, 16, 64, 64, 256
GATES = 3 * COUT
NCORES = 8
BL = B // NCORES          # batch rows per core = 2
LP = L + 2                # padded length (zero border at col 0 and L+1)
NB = 3                    # x buffer depth (steps of DMA lookahead)
F32 = mybir.dt.float32
F16 = mybir.dt.float16
AF = mybir.ActivationFunctionType
ALU = mybir.AluOpType


def _build_nc():
    nc = bacc.Bacc(trn_type="TRN2", target_bir_lowering=False, debug=False)

    # Per-core DRAM I/O (fp16 data; fp32 biases).
    x_d = nc.dram_tensor("x", [T, CIN, BL, L], F16, kind="ExternalInput").ap()
    h0_d = nc.dram_tensor("h0", [COUT, BL, L], F16, kind="ExternalInput").ap()
    wprz_d = nc.dram_tensor("wprz", [2 * CIN, 2 * COUT], F16, kind="ExternalInput").ap()
    w2rz_d = nc.dram_tensor("w2rz", [CIN, 2 * COUT], F16, kind="ExternalInput").ap()
    wpn_d = nc.dram_tensor("wpn", [2 * CIN, COUT], F16, kind="ExternalInput").ap()
    w2n_d = nc.dram_tensor("w2n", [CIN, COUT], F16, kind="ExternalInput").ap()
    whhrz_d = nc.dram_tensor("whhrz", [COUT, 3, 2 * COUT], F16, kind="ExternalInput").ap()
    whhn_d = nc.dram_tensor("whhn", [COUT, 3, COUT], F16, kind="ExternalInput").ap()
    brz_d = nc.dram_tensor("brz", [2 * COUT, 1], F32, kind="ExternalInput").ap()
    bihn_d = nc.dram_tensor("bihn", [COUT, 1], F32, kind="ExternalInput").ap()
    bhhn_d = nc.dram_tensor("bhhn", [COUT, 1], F32, kind="ExternalInput").ap()
    ys_d = nc.dram_tensor("ys", [T, COUT, BL, L], F16, kind="ExternalOutput").ap()

    with tile.TileContext(nc) as tc, ExitStack() as ctx:
        persist = ctx.enter_context(tc.tile_pool(name="persist", bufs=1))
        work = ctx.enter_context(tc.tile_pool(name="work", bufs=2))
        psA = ctx.enter_context(tc.tile_pool(name="psA", bufs=3, space="PSUM"))
        psB = ctx.enter_context(tc.tile_pool(name="psB", bufs=3, space="PSUM"))

        # --- one-time setup -------------------------------------------------
        # Input-side weights, taps 0+1 K-packed (rows 0-63 tap0, 64-127 tap1).
        wprz = persist.tile([2 * CIN, 2 * COUT], F16)
        w2rz = persist.tile([CIN, 2 * COUT], F16)
        wpn = persist.tile([2 * CIN, COUT], F16)
        w2n = persist.tile([CIN, COUT], F16)
        # Recurrent weights live on partitions 64-127 (h lives there too, so
        # lhsT/rhs base partitions match; frees partitions 0-63 of the rhs
        # for other tiles and puts z*h on gpsimd cores 4-7).
        whhrz = persist.tile([2 * COUT, 3, 2 * COUT], F16)
        whhn = persist.tile([2 * COUT, 3, COUT], F16)
        for t_, d_ in ((wprz, wprz_d), (w2rz, w2rz_d), (wpn, wpn_d), (w2n, w2n_d)):
            nc.sync.dma_start(out=t_[:], in_=d_)
        nc.sync.dma_start(out=whhrz[COUT:2 * COUT], in_=whhrz_d)
        nc.sync.dma_start(out=whhn[COUT:2 * COUT], in_=whhn_d)

        brz = persist.tile([2 * COUT, 1], F32)
        bihn = persist.tile([COUT, 1], F32)
        bhhn = persist.tile([COUT, 1], F32)
        nc.sync.dma_start(out=brz[:], in_=brz_d)
        nc.sync.dma_start(out=bihn[:], in_=bihn_d)
        nc.sync.dma_start(out=bhhn[:], in_=bhhn_d)

        # h state, one tile per batch row, on partitions 64-127; interior
        # cols 1..L, zero halo at cols 0 and L+1.
        hb = [persist.tile([2 * COUT, LP], F16, tag=f"h{b}", name=f"h{b}")
              for b in range(BL)]
        for b in range(BL):
            nc.vector.memset(hb[b][:], 0.0)
            nc.sync.dma_start(out=hb[b][COUT:2 * COUT, 1:L + 1], in_=h0_d[:, b, :])

        # x buffers: rows 0-63 = x shifted right by 1 col (tap0 view),
        # rows 64-127 = x (tap1 view). Col 0 of rows 0-63 must stay zero.
        xbuf = persist.tile([2 * CIN, NB, BL, LP], F16)
        nc.vector.memset(xbuf[:], 0.0)

        def dma_x(t):
            xb = xbuf[:, t % NB]
            nc.sync.dma_start(out=xb[0:CIN, :, 1:L + 1], in_=x_d[t])
            nc.sync.dma_start(out=xb[CIN:2 * CIN, :, 0:L], in_=x_d[t])

        # --- PE warmup: ~16 dense N=512 matmuls flip the HAM gate ----------
        warm = persist.tile([128, 512], F16)
        nc.vector.memset(warm[:], 0.0)
        wps = psA.tile([128, BL, L], F32, tag="bankA")
        for _ in range(16):
            nc.tensor.matmul(wps[:], warm[:, 0:128], warm[:, 0:2 * L],
                             start=True, stop=True)

        for t in range(min(NB - 1, T)):
            dma_x(t)

        def wih_mms(t):
            """Input-side convs for step t, both batch rows: 4 MMs at N=512.

            The two skewed chains share these banks; Tile's PSUM bank-hazard
            tracking serializes row 1's recurrent MMs behind row 0's reads,
            which is exactly the skew we want anyway."""
            xb = xbuf[:, t % NB]
            bankA = psA.tile([128, BL, L], F32, tag="bankA")
            bankB = psB.tile([128, BL, L], F32, tag="bankB")
            nc.tensor.matmul(bankA[:], wprz[:], xb[:, :, 0:L],
                             start=True, stop=False)
            nc.tensor.matmul(bankA[:], w2rz[:], xb[0:CIN, :, 2:LP],
                             start=False, stop=False)
            nc.tensor.matmul(bankB[COUT:2 * COUT], wpn[:], xb[:, :, 0:L],
                             start=True, stop=False)
            nc.tensor.matmul(bankB[COUT:2 * COUT], w2n[:], xb[0:CIN, :, 2:LP],
                             start=False, stop=True)
            return bankA, bankB

        def whh_mms(t, b, bankAB):
            """Recurrent convs for (t, b): 6 MMs at N=256, need h(t-1)."""
            bankA, bankB = bankAB
            h = hb[b]
            for k in range(3):
                nc.tensor.matmul(bankA[:, b], whhrz[COUT:2 * COUT, k],
                                 h[COUT:2 * COUT, k:k + L],
                                 start=False, stop=(k == 2))
            for k in range(3):
                nc.tensor.matmul(bankB[0:COUT, b], whhn[COUT:2 * COUT, k],
                                 h[COUT:2 * COUT, k:k + L],
                                 start=(k == 0), stop=(k == 2))

        # Prime: input convs for step 0.
        banks = [None] * T
        banks[0] = wih_mms(0)

        for t in range(T):
            if t + NB - 1 < T:
                dma_x(t + NB - 1)
            for b in range(BL):
                bankA, bankB = banks[t]
                whh_mms(t, b, banks[t])
                # Prefetch next step's input convs while this chain's
                # elementwise tail runs (keeps the PE dense and warm).
                if b == 0 and t + 1 < T:
                    banks[t + 1] = wih_mms(t + 1)

                # The whole elementwise tail lives on partitions 64-127 so
                # every SBUF+SBUF operand pair shares a base partition (BIR
                # verifier requirement) and gpsimd work lands on cores 4-7.
                h = hb[b]
                rz = work.tile([2 * COUT, L], F16, tag=f"rz{b}")
                nc.scalar.activation(rz[:], bankA[:, b], AF.Sigmoid,
                                     bias=brz[:])
                # t1 = (gh_n + bhhn) * r ; t2 = t1 + i_n ; n = tanh(t2 + bihn)
                t1 = work.tile([2 * COUT, L], F16, tag=f"t1{b}")
                nc.vector.scalar_tensor_tensor(t1[COUT:2 * COUT],
                                               bankB[0:COUT, b], bhhn[:],
                                               rz[0:COUT], op0=ALU.add,
                                               op1=ALU.mult)
                t2 = work.tile([2 * COUT, L], F16, tag=f"t2{b}")
                nc.vector.tensor_add(t2[COUT:2 * COUT], t1[COUT:2 * COUT],
                                     bankB[COUT:2 * COUT, b])
                n = work.tile([2 * COUT, L], F16, tag=f"n{b}")
                nc.scalar.activation(n[COUT:2 * COUT], t2[COUT:2 * COUT],
                                     AF.Tanh, bias=bihn[:])
                # zc = 1 - z on DVE (fills the tanh wait slot); zh = z*h on
                # GPSIMD (cores 4-7; all operands on partitions 64-127).
                zc = work.tile([2 * COUT, L], F16, tag=f"zc{b}")
                nc.vector.tensor_scalar(zc[COUT:2 * COUT],
                                        rz[COUT:2 * COUT], -1.0, 1.0,
                                        op0=ALU.mult, op1=ALU.add)
                zh = work.tile([2 * COUT, L], F16, tag=f"zh{b}")
                nc.gpsimd.tensor_mul(zh[COUT:2 * COUT], rz[COUT:2 * COUT],
                                     h[COUT:2 * COUT, 1:L + 1])
                # h_new = z*h + n*zc
                nzc = work.tile([2 * COUT, L], F16, tag=f"nzc{b}")
                nc.vector.tensor_mul(nzc[COUT:2 * COUT], n[COUT:2 * COUT],
                                     zc[COUT:2 * COUT])
                nc.vector.tensor_add(h[COUT:2 * COUT, 1:L + 1],
                                     zh[COUT:2 * COUT], nzc[COUT:2 * COUT])
                nc.sync.dma_start(out=ys_d[t, :, b, :],
                                  in_=h[COUT:2 * COUT, 1:L + 1])

    nc.compile()
    return nc


_NC = None


def _get_nc():
    global _NC
    if _NC is None:
        _NC = _build_nc()
    return _NC


def _prep_in_maps(x, h0, w_ih, w_hh, b_ih, b_hh):
    w_ih = np.asarray(w_ih, np.float32)   # [GATES, CIN, 3]
    w_hh = np.asarray(w_hh, np.float32)
    b_ih = np.asarray(b_ih, np.float32)
    b_hh = np.asarray(b_hh, np.float32)

    # lhsT layouts: [K, M] with K = input-channel rows, M = gate cols.
    def packed(w, g0, g1):   # taps 0,1 stacked on K
        return np.concatenate(
            [np.transpose(w[g0:g1, :, 0], (1, 0)),
             np.transpose(w[g0:g1, :, 1], (1, 0))], axis=0).astype(np.float16)

    def tap2(w, g0, g1):
        return np.ascontiguousarray(
            np.transpose(w[g0:g1, :, 2], (1, 0))).astype(np.float16)

    wprz = packed(w_ih, 0, 2 * COUT)
    wpn = packed(w_ih, 2 * COUT, GATES)
    w2rz = tap2(w_ih, 0, 2 * COUT)
    w2n = tap2(w_ih, 2 * COUT, GATES)
    # Recurrent weights per tap: [COUT, 3, M]
    whhrz = np.ascontiguousarray(
        np.transpose(w_hh[0:2 * COUT], (1, 2, 0))).astype(np.float16)
    whhn = np.ascontiguousarray(
        np.transpose(w_hh[2 * COUT:GATES], (1, 2, 0))).astype(np.float16)

    brz = (b_ih[:2 * COUT] + b_hh[:2 * COUT]).reshape(2 * COUT, 1)
    bihn = b_ih[2 * COUT:].reshape(COUT, 1)
    bhhn = b_hh[2 * COUT:].reshape(COUT, 1)

    x = np.asarray(x, np.float32).astype(np.float16)
    h0 = np.asarray(h0, np.float32).astype(np.float16)
    in_maps = []
    for c in range(NCORES):
        xs = np.ascontiguousarray(
            np.transpose(x[:, c * BL:(c + 1) * BL], (0, 2, 1, 3)))
        h0s = np.ascontiguousarray(
            np.transpose(h0[c * BL:(c + 1) * BL], (1, 0, 2)))
        in_maps.append({
            "x": xs, "h0": h0s,
            "wprz": wprz, "w2rz": w2rz, "wpn": wpn, "w2n": w2n,
            "whhrz": whhrz, "whhn": whhn,
            "brz": brz, "bihn": bihn, "bhhn": bhhn,
        })
    return in_maps


def kernel(x, h0, w_ih, w_hh, b_ih, b_hh):
    nc = _get_nc()
    in_maps = _prep_in_maps(x, h0, w_ih, w_hh, b_ih, b_hh)
    res = run_bass_kernel_spmd(nc, in_maps, list(range(NCORES)))
    ys = np.empty((T, B, COUT, L), np.float32)
    for c in range(NCORES):
        ys[:, c * BL:(c + 1) * BL] = np.transpose(
            res.results[c]["ys"].astype(np.float32), (0, 2, 1, 3))
    return ys
